# revision 1
# baseline (speedup 1.0000x reference)
"""Trainium2 Bass kernel for CustomFlaxViTSelfAttention (B=64, S=577, D=768, H=12).

Strategy: data-parallel over batch across 8 NeuronCores (8 batches/core).
Per core, per batch (all matmuls bf16 on the PE, fp32 PSUM accumulate):
  - X^T tiles loaded via DMA xbar transpose (host pre-pads S 577->640 and
    casts hidden_states to bf16).
  - qT/kT computed transposed ([n, s], head-paired [128, S] tiles); V (and
    K for uniform heads) computed natural ([t, n]).
  - Heads are host-permuted to [uniform..., relu...]; output unpermutes at
    evict time (per-head dest column block), so no host-side gather.
  - relu branch: scoresT[t, s] per head -> relu -> bf16 SBUF; PV matmul with
    a literal ones-column appended to V gives both O and the L1 rowsum in
    one PSUM tile; an eps row memset into the relu'd scores makes the
    rowsum come out as (sum + 1e-5) exactly; normalize with per-partition
    reciprocal scale during PSUM->SBUF evict.
  - uniform branch: O_u = (q/8) @ (K^T V / S) -- rank-64 shortcut, no SxS.
"""

import sys

sys.path.insert(0, "/opt/trn_rl_repo")

import numpy as np
import ml_dtypes

import concourse.bass as bass  # noqa: F401  (import keeps bass registered)
import concourse.mybir as mybir
import concourse.tile as tile
from concourse import bacc
from concourse.bass_utils import run_bass_kernel_spmd

B, S, D, H, HD = 64, 577, 768, 12, 64
S_PAD = 640                  # dma_start_transpose needs free dim % 128 == 0
N_CORES = 8
B_PC = B // N_CORES
KT = D // 128                # 6 contraction tiles
NT = (S + 127) // 128        # 5 token tiles (128,128,128,128,65)
EPS = 1e-5
BF16 = mybir.dt.bfloat16
F32 = mybir.dt.float32
Copy = mybir.ActivationFunctionType.Copy
Relu = mybir.ActivationFunctionType.Relu

S_CHUNKS = [(i * 128, min(128, S - i * 128)) for i in range(NT)]     # M-dim tiles
N_CHUNKS = [(0, 512), (512, S - 512)]                                # PSUM-bank N tiles


class _Alt:
    """Round-robin DVE/ACT so elementwise work splits across both engines.

    dve_share: out of 5 consecutive ops, how many go to DVE (DVE is ~1.4x
    faster than ACT for fp32-PSUM -> bf16 evicts).
    """

    def __init__(self, nc, dve_share=None):
        self.nc, self.i = nc, 0
        self.pat = ([True, False] if not dve_share else
                    [k * dve_share % 5 < dve_share for k in range(5)])

    def _dve(self):
        self.i += 1
        return self.pat[self.i % len(self.pat)]

    def copy(self, out, in_):
        if self._dve():
            self.nc.vector.tensor_copy(out, in_)
        else:
            self.nc.scalar.activation(out, in_, Copy)

    def relu(self, out, in_):
        if self._dve():
            self.nc.vector.tensor_scalar_max(out, in_, 0.0)
        else:
            self.nc.scalar.activation(out, in_, Relu)

    def scale(self, out, in_, scale_ap):
        if self._dve():
            self.nc.vector.tensor_scalar_mul(out, in_, scale_ap)
        else:
            self.nc.scalar.activation(out, in_, Copy, scale=scale_ap)

    def scale_const(self, out, in_, c):
        if self._dve():
            self.nc.vector.tensor_scalar_mul(out, in_, float(c))
        else:
            self.nc.scalar.activation(out, in_, Copy, scale=float(c))


def _groups(n, cap):
    """Split range(n) into chunks of size <= cap."""
    out, i = [], 0
    while i < n:
        out.append(list(range(i, min(i + cap, n))))
        i += cap
    return out


def build(mask, b_pc=B_PC, stage=5, repeat=1, loop_repeat=1, pair=False,
          bf16_scores=False, bufsA=3, bufsS=2, bufsO=2, rcap=6, uo_early=False,
          ilv=False, rlbufs=1, dve_share=0):
    """Build the per-core SPMD program. mask: tuple of 12 bools (True=relu).

    stage (debug bisection): 1=projections, 2=+G, 3=+scores, 4=+relu-out, 5=full.
    repeat: run the whole batch loop N times (timing: slope over N cancels
    per-launch dispatch overhead).
    bf16_scores: scores matmul writes bf16 PSUM (1 bank, single MM over S,
    2x-mode relu evicts). PSUM banks: 2*bufsA + bf16_scores*bufsS + bufsO <= 8.
    """
    uniform = [h for h in range(H) if not mask[h]]
    relu_heads = [h for h in range(H) if mask[h]]
    perm = uniform + relu_heads          # processed order -> original head
    nu, nr = len(uniform), len(relu_heads)

    nc = bacc.Bacc("TRN2", target_bir_lowering=False, debug=False,
                   num_devices=N_CORES)
    hs = nc.dram_tensor("hs", [b_pc, S_PAD, D], BF16, kind="ExternalInput")
    wq_d = nc.dram_tensor("wq", [D, D], BF16, kind="ExternalInput")
    wk_d = nc.dram_tensor("wk", [D, D], BF16, kind="ExternalInput")
    wv_d = nc.dram_tensor("wv", [D, D], BF16, kind="ExternalInput")
    out_d = nc.dram_tensor("out", [b_pc, S, D], F32, kind="ExternalOutput")

    # kT M-tiles: 128-col blocks (aligned with qT pairing parity) that touch
    # the relu block [64*nu, 768).
    kt_mtiles = [m for m in range(KT) if 128 * m + 128 > 64 * nu] if nr else []

    ugroups = _groups(nu, 6)   # uniform-head groups (PSUM: 64*6*4B <= 1 bank)
    rgroups = _groups(nr, rcap)  # relu-head groups (PSUM: 65*6*4B <= 1 bank)

    with (
        tile.TileContext(nc) as tc,
        tc.tile_pool(name="w", bufs=1) as pw,
        tc.tile_pool(name="x", bufs=2) as px,
        tc.tile_pool(name="qkv", bufs=2) as pqkv,
        tc.tile_pool(name="rl", bufs=1) as prl,
        tc.tile_pool(name="o", bufs=2) as po,
        tc.tile_pool(name="psA", bufs=bufsA, space="PSUM") as psA,
        tc.tile_pool(name="psO", bufs=bufsO, space="PSUM") as psO,
        tc.tile_pool(name="psS", bufs=bufsS, space="PSUM") as psSpool,
    ):
        psS = psSpool if bf16_scores else psA
        psG = psO
        alt = _Alt(nc, dve_share=dve_share)
        import contextlib
        loop_ctx = tc.For_i(0, loop_repeat, 1) if loop_repeat > 1 else contextlib.nullcontext()

        # ---- weights, loaded once: [128 k-part, KT k-tile, 768 out-col] ----
        wq = pw.tile([128, KT, D], BF16, tag="wq")
        wk = pw.tile([128, KT, D], BF16, tag="wk")
        wv = pw.tile([128, KT, D], BF16, tag="wv")
        for wt, wd in ((wq, wq_d), (wk, wk_d), (wv, wv_d)):
            nc.sync.dma_start(out=wt[:], in_=wd[:].rearrange("(kt k) n -> k kt n", k=128))

        with loop_ctx:
         for b in [bb for _ in range(repeat) for bb in range(b_pc)]:
            # ---- X^T via xbar transpose: 6 tiles [128 k, 640 s] bf16 ----
            xts = []
            for j in range(KT):
                xt = px.tile([128, S_PAD], BF16, tag=f"xt{j}")
                nc.sync.dma_start_transpose(xt[:], hs[b, :, 128 * j:128 * (j + 1)])
                xts.append(xt)

            # ---- qT: 6 head-paired tiles [128, S] bf16 (head h -> tile h//2, half h%2)
            qts = []
            for m in (range(KT) if stage >= 1 else []):
                ps = psA.tile([128, S], F32, tag="big")
                for k in range(KT):
                    for s0, w in N_CHUNKS:
                        nc.tensor.matmul(ps[:, s0:s0 + w],
                                         wq[:, k, 128 * m:128 * (m + 1)],
                                         xts[k][:, s0:s0 + w],
                                         start=(k == 0), stop=(k == KT - 1))
                qt = pqkv.tile([128, S], BF16, tag=f"qt{m}")
                alt.copy(qt[:, :], ps[:, :])
                qts.append(qt)

            # ---- kT for relu heads: same pairing as qT ----
            kts = {}   # m -> [128, S] bf16 tile
            for m in (kt_mtiles if stage >= 1 else []):
                ps = psA.tile([128, S], F32, tag="big")
                for k in range(KT):
                    for s0, w in N_CHUNKS:
                        nc.tensor.matmul(ps[:, s0:s0 + w],
                                         wk[:, k, 128 * m:128 * (m + 1)],
                                         xts[k][:, s0:s0 + w],
                                         start=(k == 0), stop=(k == KT - 1))
                kt = pqkv.tile([128, S], BF16, tag=f"kt{m}")
                alt.copy(kt[:, :], ps[:, :])
                kts[m] = kt

            # ---- V natural [t, n] + literal ones-columns: [128, 12, 65] bf16
            vts = []
            for t in (range(NT) if stage >= 1 else []):
                ps = psA.tile([128, D], F32, tag="big")
                for k in range(KT):
                    for c0, cw in ((0, 512), (512, 256)):
                        nc.tensor.matmul(ps[:, c0:c0 + cw],
                                         xts[k][:, 128 * t:128 * (t + 1)],
                                         wv[:, k, c0:c0 + cw],
                                         start=(k == 0), stop=(k == KT - 1))
                vt = px.tile([128, H, HD + 2], BF16, tag=f"vt{t}")
                alt.copy(vt[:, :, 0:HD], ps[:].rearrange("p (h d) -> p h d", d=HD))
                nc.gpsimd.memset(vt[:, :, HD:HD + 1], 1.0)
                vts.append(vt)

            # ---- K natural for uniform heads (per group) + G = K^T V / S ----
            g_tiles = {}   # u -> (tile, half) ; paired like qT parity
            for grp in (ugroups if stage >= 2 else []):
                gw = 64 * len(grp)
                knats = []
                for t in range(NT):
                    ps = psO.tile([128, 512], F32, tag="O")
                    for k in range(KT):
                        nc.tensor.matmul(ps[:, 0:gw],
                                         xts[k][:, 128 * t:128 * (t + 1)],
                                         wk[:, k, 64 * grp[0]:64 * grp[0] + gw],
                                         start=(k == 0), stop=(k == KT - 1))
                    kn = pqkv.tile([128, 384], BF16, tag=f"kn{t}")
                    alt.copy(kn[:, 0:gw], ps[:, 0:gw])
                    knats.append(kn)
                for ui, u in enumerate(grp):
                    half = u % 2
                    gp = psG.tile([128, 512], F32, tag="O")
                    for t in range(NT):
                        nc.tensor.matmul(gp[64 * half:64 * half + 64, 0:HD],
                                         knats[t][:, 64 * ui:64 * ui + 64],
                                         vts[t][:, u, 0:HD],
                                         start=(t == 0), stop=(t == NT - 1))
                    gt = po.tile([128, HD], BF16, tag=f"g{u // 2}")
                    alt.scale_const(gt[64 * half:64 * half + 64, :],
                                    gp[64 * half:64 * half + 64, 0:HD], 1.0 / S)
                    g_tiles[u] = gt

            # ---- uniform outputs early (off the tail critical path) ----
            stgs = {}
            if uo_early and stage >= 5:
                for si, (s0, sw) in enumerate(S_CHUNKS):
                    stg = po.tile([128, D], F32, tag=f"st{si}")
                    stgs[si] = stg
                    for grp in ugroups:
                        for par in (0, 1):
                            pgrp = [u for u in grp if u % 2 == par]
                            if not pgrp:
                                continue
                            op = psO.tile([128, 512], F32, tag="O")
                            for ui, u in enumerate(pgrp):
                                nc.tensor.matmul(op[0:sw, 64 * ui:64 * ui + 64],
                                                 qts[u // 2][64 * par:64 * par + 64, s0:s0 + sw],
                                                 g_tiles[u][64 * par:64 * par + 64, :],
                                                 start=True, stop=True)
                            for ui, u in enumerate(pgrp):
                                g0 = 64 * perm[u]
                                alt.copy(stg[0:sw, g0:g0 + HD],
                                         op[0:sw, 64 * ui:64 * ui + 64])

            # ---- relu heads: scoresT -> relu(bf16) -> PV with ones-col ----
            # Dedicated 2-bank scores PSUM pool (psS): scores go evict-bound,
            # but next-batch projections keep the PE busy via psA. Each evict
            # splits big/small chunks across DVE and ACT in parallel.
            rls = {}   # (r, t) -> relu'd scoresT tile [t-part, s-free]
            sc_dt = BF16 if bf16_scores else F32
            sc_tag = "sc" if bf16_scores else "big"
            sc_chunks = [(0, S)] if bf16_scores else N_CHUNKS
            if ilv and stage >= 5:
                # Interleaved scores+PV per relu group: spreads the PSUM-evict
                # load (the structural bottleneck) across the attention span.
                assert uo_early, "ilv requires uo_early (stgs pre-built)"
                for gi, grp in enumerate(rgroups):
                    by_m = {}
                    for r in grp:
                        h = nu + r
                        by_m.setdefault(h // 2, []).append(h % 2)
                    for t in range(NT):
                        t0, tw = 128 * t, min(128, S - 128 * t)
                        for m, halves in sorted(by_m.items()):
                            pss = {half: psS.tile([128, S], sc_dt, tag=sc_tag,
                                                  name=f"scps{half}")
                                   for half in halves}
                            for s0, w in sc_chunks:
                                for half in halves:
                                    nc.tensor.matmul(pss[half][0:tw, s0:s0 + w],
                                                     kts[m][64 * half:64 * half + 64, t0:t0 + tw],
                                                     qts[m][64 * half:64 * half + 64, s0:s0 + w],
                                                     start=True, stop=True)
                            for half in halves:
                                r = 2 * m + half - nu
                                rl = prl.tile([tw, S], BF16,
                                              tag=f"rl{r % rcap}_{t}",
                                              bufs=rlbufs)
                                alt.relu(rl[0:tw, :], pss[half][0:tw, :])
                                rls[(r, t)] = rl
                    for si, (s0, sw) in enumerate(S_CHUNKS):
                        op = psO.tile([128, 512], F32, tag="O")
                        for ri, r in enumerate(grp):
                            h = nu + r
                            for t in range(NT):
                                kk = 128 if t < NT - 1 else S - 512
                                nc.tensor.matmul(op[0:sw, 65 * ri:65 * ri + 65],
                                                 rls[(r, t)][0:kk, s0:s0 + sw],
                                                 vts[t][0:kk, h, 0:HD + 1],
                                                 start=(t == 0), stop=(t == NT - 1))
                        rec = po.tile([128, 8], F32, tag="rec")
                        ng = len(grp)
                        op3 = op[0:sw, 0:65 * ng].rearrange("p (r c) -> p r c", c=65)
                        nc.vector.tensor_scalar_add(rec[0:sw, 0:ng], op3[:, :, HD], EPS)
                        nc.vector.reciprocal(rec[0:sw, 0:ng], rec[0:sw, 0:ng])
                        for ri, r in enumerate(grp):
                            g0 = 64 * perm[nu + r]
                            alt.scale(stgs[si][0:sw, g0:g0 + HD],
                                      op[0:sw, 65 * ri:65 * ri + HD],
                                      rec[0:sw, ri:ri + 1])
                for si, (s0, sw) in enumerate(S_CHUNKS):
                    nc.gpsimd.dma_start(out=out_d[b, s0:s0 + sw, :],
                                        in_=stgs[si][0:sw, :])
            elif stage >= 3 and not pair:
                for r in range(nr):
                    h = nu + r
                    m, half = h // 2, h % 2
                    for t in range(NT):
                        t0, tw = 128 * t, min(128, S - 128 * t)
                        ps = psS.tile([128, S], sc_dt, tag=sc_tag)
                        for s0, w in sc_chunks:
                            nc.tensor.matmul(ps[0:tw, s0:s0 + w],
                                             kts[m][64 * half:64 * half + 64, t0:t0 + tw],
                                             qts[m][64 * half:64 * half + 64, s0:s0 + w],
                                             start=True, stop=True)
                        rl = prl.tile([tw, S], BF16, tag=f"rl{r}_{t}")
                        alt.relu(rl[0:tw, :], ps[0:tw, :])
                        rls[(r, t)] = rl
            elif stage >= 3:
                # paired halves: concurrent PE row groups, different PSUM
                # slots, evicts split across DVE/ACT.
                for m in kt_mtiles:
                    halves = [p for p in (0, 1) if 2 * m + p >= nu]
                    for t in range(NT):
                        t0, tw = 128 * t, min(128, S - 128 * t)
                        pss = {}
                        for half in halves:
                            sc_ps = psS.tile([128, S], sc_dt, tag=sc_tag)
                            pss[half] = sc_ps
                        for s0, w in sc_chunks:
                            for half in halves:
                                nc.tensor.matmul(pss[half][0:tw, s0:s0 + w],
                                                 kts[m][64 * half:64 * half + 64, t0:t0 + tw],
                                                 qts[m][64 * half:64 * half + 64, s0:s0 + w],
                                                 start=True, stop=True)
                        for hi, half in enumerate(halves):
                            r = 2 * m + half - nu
                            rl = prl.tile([tw, S], BF16, tag=f"rl{r}_{t}")
                            if (hi + t) % 2:
                                nc.vector.tensor_scalar_max(rl[0:tw, :], pss[half][0:tw, :], 0.0)
                            else:
                                nc.scalar.activation(rl[0:tw, :], pss[half][0:tw, :], Relu)
                            rls[(r, t)] = rl

            # ---- outputs per s-tile ----
            for si, (s0, sw) in enumerate(S_CHUNKS if not ilv else []):
                stg = stgs[si] if si in stgs else po.tile([128, D], F32, tag=f"st{si}")
                if stage < 5:
                    nc.vector.memset(stg[:], 0.0)

                for grp in (rgroups if stage >= 4 else []):
                    op = psO.tile([128, 512], F32, tag="O")
                    for ri, r in enumerate(grp):
                        h = nu + r
                        for t in range(NT):
                            kk = 128 if t < NT - 1 else S - 512
                            nc.tensor.matmul(op[0:sw, 65 * ri:65 * ri + 65],
                                             rls[(r, t)][0:kk, s0:s0 + sw],
                                             vts[t][0:kk, h, 0:HD + 1],
                                             start=(t == 0), stop=(t == NT - 1))
                    rec = po.tile([128, 8], F32, tag="rec")
                    ng = len(grp)
                    op3 = op[0:sw, 0:65 * ng].rearrange("p (r c) -> p r c", c=65)
                    nc.vector.tensor_scalar_add(rec[0:sw, 0:ng], op3[:, :, HD], EPS)
                    nc.vector.reciprocal(rec[0:sw, 0:ng], rec[0:sw, 0:ng])
                    for ri, r in enumerate(grp):
                        g0 = 64 * perm[nu + r]
                        alt.scale(stg[0:sw, g0:g0 + HD],
                                  op[0:sw, 65 * ri:65 * ri + HD],
                                  rec[0:sw, ri:ri + 1])

                # NB: matmuls with different PE row groups (parity halves) run
                # concurrently -> their drains must target different PSUM
                # banks, so split the uniform heads by parity.
                for grp in (ugroups if stage >= 5 and not uo_early else []):
                    for par in (0, 1):
                        pgrp = [u for u in grp if u % 2 == par]
                        if not pgrp:
                            continue
                        op = psO.tile([128, 512], F32, tag="O")
                        for ui, u in enumerate(pgrp):
                            nc.tensor.matmul(op[0:sw, 64 * ui:64 * ui + 64],
                                             qts[u // 2][64 * par:64 * par + 64, s0:s0 + sw],
                                             g_tiles[u][64 * par:64 * par + 64, :],
                                             start=True, stop=True)
                        for ui, u in enumerate(pgrp):
                            g0 = 64 * perm[u]
                            alt.copy(stg[0:sw, g0:g0 + HD],
                                     op[0:sw, 64 * ui:64 * ui + 64])

                nc.gpsimd.dma_start(out=out_d[b, s0:s0 + sw, :], in_=stg[0:sw, :])

    nc.compile()
    return nc


_CACHE = {}


def _get_nc(mask, b_pc=B_PC):
    key = (mask, b_pc)
    if key not in _CACHE:
        _CACHE[key] = build(mask, b_pc)
    return _CACHE[key]


def prep_inputs(hidden_states, Wq, Wk, Wv, mask):
    """Host-side prep: head permutation, 1/sqrt(hd) fold, bf16 cast, S pad."""
    uniform = [h for h in range(H) if not mask[h]]
    relu_heads = [h for h in range(H) if mask[h]]
    perm = uniform + relu_heads
    cols = np.concatenate([np.arange(64 * h, 64 * h + 64) for h in perm])
    wq_p = np.ascontiguousarray(
        (np.asarray(Wq, np.float32)[:, cols] * 0.125).astype(ml_dtypes.bfloat16))
    wk_p = np.ascontiguousarray(
        np.asarray(Wk, np.float32)[:, cols].astype(ml_dtypes.bfloat16))
    wv_p = np.ascontiguousarray(
        np.asarray(Wv, np.float32)[:, cols].astype(ml_dtypes.bfloat16))
    hsf = np.asarray(hidden_states, np.float32)
    hs_p = np.zeros((hsf.shape[0], S_PAD, D), ml_dtypes.bfloat16)
    hs_p[:, :S] = hsf.astype(ml_dtypes.bfloat16)
    return hs_p, wq_p, wk_p, wv_p


def make_in_maps(hidden_states, Wq, Wk, Wv, mask, b_pc=B_PC):
    hs_p, wq_p, wk_p, wv_p = prep_inputs(hidden_states, Wq, Wk, Wv, mask)
    n_shards = hs_p.shape[0] // b_pc
    return [
        {"hs": hs_p[c * b_pc:(c + 1) * b_pc], "wq": wq_p, "wk": wk_p, "wv": wv_p}
        for c in range(n_shards)
    ]


def kernel(hidden_states, Wq, bq, Wk, bk, Wv, bv, head_mask, layer_count=None, **_):
    for bias in (bq, bk, bv):
        assert not np.any(np.asarray(bias)), "nonzero qkv biases unsupported"
    mask = tuple(bool(x) for x in np.asarray(head_mask).reshape(-1))
    assert len(mask) == H

    nc = _get_nc(mask)
    in_maps = make_in_maps(hidden_states, Wq, Wk, Wv, mask)
    res = run_bass_kernel_spmd(nc, in_maps, list(range(N_CORES)))
    out = np.concatenate([res.results[c]["out"] for c in range(N_CORES)], axis=0)
    return np.ascontiguousarray(out.astype(np.float32))



# revision 11
# speedup vs baseline: 1.1082x; 1.1082x over previous
"""Trainium2 Bass kernel for CustomFlaxViTSelfAttention (B=64, S=577, D=768, H=12).

Strategy: data-parallel over batch across 8 NeuronCores (8 batches/core).
Per core, per batch (all matmuls bf16 on the PE, fp32 PSUM accumulate):
  - X^T tiles pre-transposed ON HOST (hs shipped as [b, D, S_PAD] bf16,
    zero-padded S 577->640) -> plain fast DMAs, no xbar transpose.
  - Weights DMA'd in per-k-tile chunks so the first projection matmul can
    start as soon as chunk 0 + xts[0] land.
  - qT/kT computed transposed ([n, s], head-paired [128, S] tiles); V (and
    K for uniform heads) computed natural ([t, n]).
  - Heads are host-permuted to [uniform..., relu...]; output stays in
    processed order ON DEVICE (bf16) and is un-permuted + upcast on host.
  - relu branch: scoresT[t, s] per head -> relu -> bf16 SBUF; PV matmul with
    a literal ones-column appended to V gives both O and the L1 rowsum in
    one PSUM tile; normalize with a single broadcast tensor_mul per s-tile
    (rec[s, head] broadcast over the 64 head cols, stride-0 AP).
  - uniform branch: O_u = (q/8) @ (K^T V / S) -- rank-64 shortcut, no SxS;
    evicted with one strided merged copy per (parity, s-tile).
"""

import sys

sys.path.insert(0, "/opt/trn_rl_repo")

import numpy as np
import ml_dtypes

import concourse.bass as bass  # noqa: F401  (import keeps bass registered)
import concourse.mybir as mybir
import concourse.tile as tile
from concourse import bacc
from concourse.bass_utils import run_bass_kernel_spmd

B, S, D, H, HD = 64, 577, 768, 12, 64
S_PAD = 640                  # zero-padded token dim (junk-free V/knat tails)
N_CORES = 8
B_PC = B // N_CORES
KT = D // 128                # 6 contraction tiles
NT = (S + 127) // 128        # 5 token tiles (128,128,128,128,65)
EPS = 1e-5
BF16 = mybir.dt.bfloat16
F32 = mybir.dt.float32
Copy = mybir.ActivationFunctionType.Copy
Relu = mybir.ActivationFunctionType.Relu

S_CHUNKS = [(i * 128, min(128, S - i * 128)) for i in range(NT)]     # M-dim tiles
N_CHUNKS = [(0, 512), (512, S - 512)]                                # PSUM-bank N tiles


class _Alt:
    """Round-robin DVE/ACT so elementwise work splits across both engines."""

    def __init__(self, nc, dve_share=None):
        self.nc, self.i = nc, 0
        self.pat = ([True, False] if not dve_share else
                    [k * dve_share % 5 < dve_share for k in range(5)])

    def _dve(self):
        self.i += 1
        return self.pat[self.i % len(self.pat)]

    def copy(self, out, in_):
        if self._dve():
            self.nc.vector.tensor_copy(out, in_)
        else:
            self.nc.scalar.activation(out, in_, Copy)

    def relu(self, out, in_):
        if self._dve():
            self.nc.vector.tensor_scalar_max(out, in_, 0.0)
        else:
            self.nc.scalar.activation(out, in_, Relu)

    def scale(self, out, in_, scale_ap):
        if self._dve():
            self.nc.vector.tensor_scalar_mul(out, in_, scale_ap)
        else:
            self.nc.scalar.activation(out, in_, Copy, scale=scale_ap)

    def scale_const(self, out, in_, c):
        if self._dve():
            self.nc.vector.tensor_scalar_mul(out, in_, float(c))
        else:
            self.nc.scalar.activation(out, in_, Copy, scale=float(c))


def _groups(n, cap):
    """Split range(n) into chunks of size <= cap."""
    out, i = [], 0
    while i < n:
        out.append(list(range(i, min(i + cap, n))))
        i += cap
    return out


def build(mask, b_pc=B_PC, stage=5, repeat=1, loop_repeat=1, pair=False,
          bf16_scores=False, bufsA=3, bufsS=2, bufsO=2, rcap=6, uo_early=True,
          ilv=False, rlbufs=1, dve_share=0, split_relu=False):
    """Build the per-core SPMD program. mask: tuple of 12 bools (True=relu).

    loop_repeat: device-side For_i around the whole batch loop (timing).
    """
    uniform = [h for h in range(H) if not mask[h]]
    relu_heads = [h for h in range(H) if mask[h]]
    nu, nr = len(uniform), len(relu_heads)

    nc = bacc.Bacc("TRN2", target_bir_lowering=False, debug=False,
                   num_devices=N_CORES)
    hs = nc.dram_tensor("hs", [b_pc, D, S_PAD], BF16, kind="ExternalInput")
    wq_d = nc.dram_tensor("wq", [D, D], BF16, kind="ExternalInput")
    wk_d = nc.dram_tensor("wk", [D, D], BF16, kind="ExternalInput")
    wv_d = nc.dram_tensor("wv", [D, D], BF16, kind="ExternalInput")
    out_d = nc.dram_tensor("out", [b_pc, S, D], BF16, kind="ExternalOutput")

    # kT M-tiles: 128-col blocks (aligned with qT pairing parity) that touch
    # the relu block [64*nu, 768).
    kt_mtiles = [m for m in range(KT) if 128 * m + 128 > 64 * nu] if nr else []

    ugroups = _groups(nu, 6)   # uniform-head groups (PSUM: 64*6*4B <= 1 bank)
    rgroups = _groups(nr, rcap)  # relu-head groups (PSUM: 65*6*4B <= 1 bank)

    with (
        tile.TileContext(nc) as tc,
        tc.tile_pool(name="w", bufs=1) as pw,
        tc.tile_pool(name="x", bufs=2) as px,
        tc.tile_pool(name="qkv", bufs=2) as pqkv,
        tc.tile_pool(name="rl", bufs=1) as prl,
        tc.tile_pool(name="o", bufs=2) as po,
        tc.tile_pool(name="psA", bufs=bufsA, space="PSUM") as psA,
        tc.tile_pool(name="psO", bufs=bufsO, space="PSUM") as psO,
    ):
        psS = psA
        psG = psO
        alt = _Alt(nc, dve_share=dve_share)
        import contextlib
        loop_ctx = tc.For_i(0, loop_repeat, 1) if loop_repeat > 1 else contextlib.nullcontext()

        # ---- weights, loaded once: [128 k-part, KT k-tile, 768 out-col] ----
        # On the ACT HWDGE queue (X tiles go via SP) so neither waits on the
        # other; per-weight split (k0 | k1..5) so the first matmul can start
        # as soon as the first chunk lands.
        wq = pw.tile([128, KT, D], BF16, tag="wq")
        wk = pw.tile([128, KT, D], BF16, tag="wk")
        wv = pw.tile([128, KT, D], BF16, tag="wv")
        for wt, wd in ((wq, wq_d), (wk, wk_d), (wv, wv_d)):
            nc.scalar.dma_start(
                out=wt[:, 0, :], in_=wd[0:128, :])
            nc.scalar.dma_start(
                out=wt[:, 1:KT, :],
                in_=wd[128:D, :].rearrange("(kt k) n -> k kt n", k=128))

        with loop_ctx:
         for b in [bb for _ in range(repeat) for bb in range(b_pc)]:
            # ---- X^T tiles (host pre-transposed): [128 k, KT, 640 s] bf16 ----
            # one tile, two DMAs (k0 | k1..5) on the SP HWDGE queue
            xts = px.tile([128, KT, S_PAD], BF16, tag="xts")
            nc.sync.dma_start(out=xts[:, 0, :], in_=hs[b, 0:128, :])
            nc.sync.dma_start(
                out=xts[:, 1:KT, :],
                in_=hs[b, 128:D, :].rearrange("(kt k) s -> k kt s", k=128))

            # ---- qT: 6 head-paired tiles [128, S] bf16 (head h -> tile h//2, half h%2)
            qts = []
            for m in (range(KT) if stage >= 1 else []):
                ps = psA.tile([128, S], F32, tag="big")
                for k in range(KT):
                    for s0, w in N_CHUNKS:
                        nc.tensor.matmul(ps[:, s0:s0 + w],
                                         wq[:, k, 128 * m:128 * (m + 1)],
                                         xts[:, k, s0:s0 + w],
                                         start=(k == 0), stop=(k == KT - 1))
                qt = pqkv.tile([128, S], BF16, tag=f"qt{m}")
                alt.copy(qt[:, :], ps[:, :])
                qts.append(qt)

            # ---- kT for relu heads: same pairing as qT ----
            kts = {}   # m -> [128, S] bf16 tile
            for m in (kt_mtiles if stage >= 1 else []):
                ps = psA.tile([128, S], F32, tag="big")
                for k in range(KT):
                    for s0, w in N_CHUNKS:
                        nc.tensor.matmul(ps[:, s0:s0 + w],
                                         wk[:, k, 128 * m:128 * (m + 1)],
                                         xts[:, k, s0:s0 + w],
                                         start=(k == 0), stop=(k == KT - 1))
                kt = pqkv.tile([128, S], BF16, tag=f"kt{m}")
                alt.copy(kt[:, :], ps[:, :])
                kts[m] = kt

            # ---- V natural [t, n] + literal ones-columns: [128, 12, 66] bf16
            vts = []
            for t in (range(NT) if stage >= 1 else []):
                ps = psA.tile([128, D], F32, tag="big")
                for k in range(KT):
                    for c0, cw in ((0, 512), (512, 256)):
                        nc.tensor.matmul(ps[:, c0:c0 + cw],
                                         xts[:, k, 128 * t:128 * (t + 1)],
                                         wv[:, k, c0:c0 + cw],
                                         start=(k == 0), stop=(k == KT - 1))
                vt = px.tile([128, H, HD + 2], BF16, tag=f"vt{t}")
                alt.copy(vt[:, :, 0:HD], ps[:].rearrange("p (h d) -> p h d", d=HD))
                nc.gpsimd.memset(vt[:, :, HD:HD + 1], 1.0)
                vts.append(vt)

            # ---- K natural for uniform heads (per group) + G = K^T V / S ----
            g_tiles = {}   # u -> (tile, half) ; paired like qT parity
            for grp in (ugroups if stage >= 2 else []):
                gw = 64 * len(grp)
                knats = []
                for t in range(NT):
                    ps = psO.tile([128, 512], F32, tag="O")
                    for k in range(KT):
                        nc.tensor.matmul(ps[:, 0:gw],
                                         xts[:, k, 128 * t:128 * (t + 1)],
                                         wk[:, k, 64 * grp[0]:64 * grp[0] + gw],
                                         start=(k == 0), stop=(k == KT - 1))
                    kn = pqkv.tile([128, 384], BF16, tag=f"kn{t}")
                    alt.copy(kn[:, 0:gw], ps[:, 0:gw])
                    knats.append(kn)
                for ui, u in enumerate(grp):
                    half = u % 2
                    gp = psG.tile([128, 512], F32, tag="O")
                    for t in range(NT):
                        nc.tensor.matmul(gp[64 * half:64 * half + 64, 0:HD],
                                         knats[t][:, 64 * ui:64 * ui + 64],
                                         vts[t][:, u, 0:HD],
                                         start=(t == 0), stop=(t == NT - 1))
                    gt = po.tile([128, HD], BF16, tag=f"g{u // 2}")
                    alt.scale_const(gt[64 * half:64 * half + 64, :],
                                    gp[64 * half:64 * half + 64, 0:HD], 1.0 / S)
                    g_tiles[u] = gt

            # ---- uniform outputs early (off the tail critical path) ----
            # merged strided evict per (parity, s-tile): dest head-blocks are
            # stride-128 in processed order (positions par, par+2, ...)
            stgs = {}
            if stage >= 5:
                for si, (s0, sw) in enumerate(S_CHUNKS):
                    stg = po.tile([128, D], BF16, tag=f"st{si}")
                    stgs[si] = stg
                    for grp in ugroups:
                        for par in (0, 1):
                            pgrp = [u for u in grp if u % 2 == par]
                            if not pgrp:
                                continue
                            op = psO.tile([128, 512], F32, tag="O")
                            for ui, u in enumerate(pgrp):
                                nc.tensor.matmul(op[0:sw, 64 * ui:64 * ui + 64],
                                                 qts[u // 2][64 * par:64 * par + 64, s0:s0 + sw],
                                                 g_tiles[u][64 * par:64 * par + 64, :],
                                                 start=True, stop=True)
                            npg = len(pgrp)
                            # contiguous-in-processed-order check (u = par, par+2, ..)
                            if all(pgrp[i] == par + 2 * i for i in range(npg)):
                                dest = stg[0:sw, :].rearrange(
                                    "p (g c) -> p g c", c=64)[:, par:par + 2 * npg:2, :]
                                alt.copy(dest,
                                         op[0:sw, 0:64 * npg].rearrange(
                                             "p (g c) -> p g c", c=64))
                            else:
                                for ui, u in enumerate(pgrp):
                                    alt.copy(stg[0:sw, 64 * u:64 * u + HD],
                                             op[0:sw, 64 * ui:64 * ui + 64])

            # ---- relu heads: scoresT -> relu(bf16) -> PV with ones-col ----
            # Last t-tile gets one extra row memset to EPS: with V's zero
            # rows beyond S and the literal ones-column, the PV rowsum comes
            # out as (sum + EPS) exactly -- no separate add before reciprocal.
            rls = {}   # (r, t) -> relu'd scoresT tile [t-part, s-free]
            if stage >= 3:
                for r in range(nr):
                    h = nu + r
                    m, half = h // 2, h % 2
                    for t in range(NT):
                        t0, tw = 128 * t, min(128, S - 128 * t)
                        ps = psS.tile([128, S], F32, tag="big")
                        for s0, w in N_CHUNKS:
                            nc.tensor.matmul(ps[0:tw, s0:s0 + w],
                                             kts[m][64 * half:64 * half + 64, t0:t0 + tw],
                                             qts[m][64 * half:64 * half + 64, s0:s0 + w],
                                             start=True, stop=True)
                        rl = prl.tile([tw, S], BF16, tag=f"rl{r}_{t}", bufs=rlbufs)
                        if split_relu:
                            nc.vector.tensor_scalar_max(rl[0:tw, 0:320], ps[0:tw, 0:320], 0.0)
                            nc.scalar.activation(rl[0:tw, 320:S], ps[0:tw, 320:S], Relu)
                        else:
                            alt.relu(rl[0:tw, :], ps[0:tw, :])
                        rls[(r, t)] = rl

            # ---- relu outputs per s-tile (uniform already in stgs) ----
            for si, (s0, sw) in enumerate(S_CHUNKS if stage >= 4 else []):
                stg = stgs[si]
                for gi, grp in enumerate(rgroups):
                    op = psO.tile([128, 512], F32, tag="O")
                    for ri, r in enumerate(grp):
                        h = nu + r
                        for t in range(NT):
                            kk = 128 if t < NT - 1 else S - 512
                            nc.tensor.matmul(op[0:sw, 65 * ri:65 * ri + 65],
                                             rls[(r, t)][0:kk, s0:s0 + sw],
                                             vts[t][0:kk, h, 0:HD + 1],
                                             start=(t == 0), stop=(t == NT - 1))
                    rec = po.tile([128, 8], F32, tag="rec")
                    ng = len(grp)
                    op3 = op[0:sw, 0:65 * ng].rearrange("p (r c) -> p r c", c=65)
                    nc.vector.tensor_scalar_add(rec[0:sw, 0:ng], op3[:, :, HD], EPS)
                    nc.vector.reciprocal(rec[0:sw, 0:ng], rec[0:sw, 0:ng])
                    # single broadcast multiply: stg[:, relu block] = O * rec
                    g0 = 64 * (nu + grp[0])
                    rec_bc = rec[0:sw, 0:ng].unsqueeze(2).broadcast_to((sw, ng, 64))
                    nc.vector.tensor_mul(
                        stg[0:sw, g0:g0 + 64 * ng].rearrange("p (r c) -> p r c", c=64),
                        op3[:, :, 0:HD], rec_bc)

                nc.sync.dma_start(out=out_d[b, s0:s0 + sw, :], in_=stg[0:sw, :])

    nc.compile()
    return nc


_CACHE = {}


def _get_nc(mask, b_pc=B_PC):
    key = (mask, b_pc)
    if key not in _CACHE:
        _CACHE[key] = build(mask, b_pc)
    return _CACHE[key]


def prep_inputs(hidden_states, Wq, Wk, Wv, mask):
    """Host-side prep: head permutation, 1/sqrt(hd) fold, bf16 cast, S pad,
    and the X -> X^T transpose (device gets [b, D, S_PAD])."""
    uniform = [h for h in range(H) if not mask[h]]
    relu_heads = [h for h in range(H) if mask[h]]
    perm = uniform + relu_heads
    cols = np.concatenate([np.arange(64 * h, 64 * h + 64) for h in perm])
    wq_p = np.ascontiguousarray(
        (np.asarray(Wq, np.float32)[:, cols] * 0.125).astype(ml_dtypes.bfloat16))
    wk_p = np.ascontiguousarray(
        np.asarray(Wk, np.float32)[:, cols].astype(ml_dtypes.bfloat16))
    wv_p = np.ascontiguousarray(
        np.asarray(Wv, np.float32)[:, cols].astype(ml_dtypes.bfloat16))
    hsf = np.asarray(hidden_states, np.float32)
    hs_p = np.zeros((hsf.shape[0], D, S_PAD), ml_dtypes.bfloat16)
    hs_p[:, :, :S] = hsf.astype(ml_dtypes.bfloat16).transpose(0, 2, 1)
    return hs_p, wq_p, wk_p, wv_p


def make_in_maps(hidden_states, Wq, Wk, Wv, mask, b_pc=B_PC):
    hs_p, wq_p, wk_p, wv_p = prep_inputs(hidden_states, Wq, Wk, Wv, mask)
    n_shards = hs_p.shape[0] // b_pc
    return [
        {"hs": hs_p[c * b_pc:(c + 1) * b_pc], "wq": wq_p, "wk": wk_p, "wv": wv_p}
        for c in range(n_shards)
    ]


def unpermute_out(out_p, mask):
    """Device output is bf16 in processed head order; un-permute + fp32."""
    uniform = [h for h in range(H) if not mask[h]]
    relu_heads = [h for h in range(H) if mask[h]]
    perm = uniform + relu_heads
    out = np.empty(out_p.shape, np.float32)
    o3 = out.reshape(*out_p.shape[:-1], H, HD)
    p3 = np.asarray(out_p).reshape(*out_p.shape[:-1], H, HD)
    for p, horig in enumerate(perm):
        o3[..., horig, :] = p3[..., p, :].astype(np.float32)
    return out


def kernel(hidden_states, Wq, bq, Wk, bk, Wv, bv, head_mask, layer_count=None, **_):
    for bias in (bq, bk, bv):
        assert not np.any(np.asarray(bias)), "nonzero qkv biases unsupported"
    mask = tuple(bool(x) for x in np.asarray(head_mask).reshape(-1))
    assert len(mask) == H

    nc = _get_nc(mask)
    in_maps = make_in_maps(hidden_states, Wq, Wk, Wv, mask)
    res = run_bass_kernel_spmd(nc, in_maps, list(range(N_CORES)))
    out_p = np.concatenate([res.results[c]["out"] for c in range(N_CORES)], axis=0)
    return np.ascontiguousarray(unpermute_out(out_p, mask))


# revision 19
# speedup vs baseline: 1.3447x; 1.2135x over previous
"""Trainium2 Bass kernel for CustomFlaxViTSelfAttention (B=64, S=577, D=768, H=12).

Strategy: data-parallel over batch across 8 NeuronCores (8 batches/core).
Per core, per batch (all matmuls bf16 on the PE, fp32 PSUM accumulate):
  - X^T tiles pre-transposed ON HOST (hs shipped as [b, D, S_PAD] bf16,
    zero-padded S 577->640) -> plain fast DMAs, no xbar transpose.
  - Weights DMA'd in per-k-tile chunks so the first projection matmul can
    start as soon as chunk 0 + xts[0] land.
  - qT/kT computed transposed ([n, s], head-paired [128, S] tiles); V (and
    K for uniform heads) computed natural ([t, n]).
  - Heads are host-permuted to [uniform..., relu...]; output stays in
    processed order ON DEVICE (bf16) and is un-permuted + upcast on host.
  - relu branch: scoresT[t, s] per head -> relu -> bf16 SBUF; PV matmul with
    a literal ones-column appended to V gives both O and the L1 rowsum in
    one PSUM tile; normalize with a single broadcast tensor_mul per s-tile
    (rec[s, head] broadcast over the 64 head cols, stride-0 AP).
  - uniform branch: O_u = (q/8) @ (K^T V / S) -- rank-64 shortcut, no SxS;
    evicted with one strided merged copy per (parity, s-tile).
"""

import sys

sys.path.insert(0, "/opt/trn_rl_repo")

import numpy as np
import ml_dtypes

import concourse.bass as bass  # noqa: F401  (import keeps bass registered)
import concourse.mybir as mybir
import concourse.tile as tile
from concourse import bacc
from concourse.bass_utils import run_bass_kernel_spmd

B, S, D, H, HD = 64, 577, 768, 12, 64
S_PAD = 640                  # zero-padded token dim (junk-free V/knat tails)
N_CORES = 8
B_PC = B // N_CORES
KT = D // 128                # 6 contraction tiles
NT = (S + 127) // 128        # 5 token tiles (128,128,128,128,65)
EPS = 1e-5
BF16 = mybir.dt.bfloat16
F32 = mybir.dt.float32
Copy = mybir.ActivationFunctionType.Copy
Relu = mybir.ActivationFunctionType.Relu

S_CHUNKS = [(i * 128, min(128, S - i * 128)) for i in range(NT)]     # M-dim tiles
N_CHUNKS = [(0, 512), (512, S - 512)]                                # PSUM-bank N tiles


class _Alt:
    """Round-robin DVE/ACT so elementwise work splits across both engines."""

    def __init__(self, nc, dve_share=None):
        self.nc, self.i = nc, 0
        self.pat = ([True, False] if not dve_share else
                    [k * dve_share % 5 < dve_share for k in range(5)])

    def _dve(self):
        self.i += 1
        return self.pat[self.i % len(self.pat)]

    def copy(self, out, in_):
        if self._dve():
            self.nc.vector.tensor_copy(out, in_)
        else:
            self.nc.scalar.activation(out, in_, Copy)

    def relu(self, out, in_):
        if self._dve():
            self.nc.vector.tensor_scalar_max(out, in_, 0.0)
        else:
            self.nc.scalar.activation(out, in_, Relu)

    def scale(self, out, in_, scale_ap):
        if self._dve():
            self.nc.vector.tensor_scalar_mul(out, in_, scale_ap)
        else:
            self.nc.scalar.activation(out, in_, Copy, scale=scale_ap)

    def scale_const(self, out, in_, c):
        if self._dve():
            self.nc.vector.tensor_scalar_mul(out, in_, float(c))
        else:
            self.nc.scalar.activation(out, in_, Copy, scale=float(c))


def _groups(n, cap):
    """Split range(n) into chunks of size <= cap."""
    out, i = [], 0
    while i < n:
        out.append(list(range(i, min(i + cap, n))))
        i += cap
    return out


def build(mask, b_pc=B_PC, stage=5, repeat=1, loop_repeat=1, pair=False,
          bf16_scores=False, bufsA=3, bufsS=2, bufsO=2, rcap=6, uo_early=True,
          ilv=False, rlbufs=1, dve_share=0, split_relu=False):
    """Build the per-core SPMD program. mask: tuple of 12 bools (True=relu).

    loop_repeat: device-side For_i around the whole batch loop (timing).
    """
    uniform = [h for h in range(H) if not mask[h]]
    relu_heads = [h for h in range(H) if mask[h]]
    nu, nr = len(uniform), len(relu_heads)

    nc = bacc.Bacc("TRN2", target_bir_lowering=False, debug=False,
                   num_devices=N_CORES)
    hs = nc.dram_tensor("hs", [b_pc, D, S_PAD], BF16, kind="ExternalInput")
    wq_d = nc.dram_tensor("wq", [D, D], BF16, kind="ExternalInput")
    wk_d = nc.dram_tensor("wk", [D, D], BF16, kind="ExternalInput")
    wv_d = nc.dram_tensor("wv", [D, D], BF16, kind="ExternalInput")
    out_d = nc.dram_tensor("out", [b_pc, S, D], BF16, kind="ExternalOutput")

    # kT M-tiles: 128-col blocks (aligned with qT pairing parity) that touch
    # the relu block [64*nu, 768).
    kt_mtiles = [m for m in range(KT) if 128 * m + 128 > 64 * nu] if nr else []

    ugroups = _groups(nu, 6)   # uniform-head groups (PSUM: 64*6*4B <= 1 bank)
    rgroups = _groups(nr, rcap)  # relu-head groups (PSUM: 65*6*4B <= 1 bank)

    with (
        tile.TileContext(nc) as tc,
        tc.tile_pool(name="w", bufs=1) as pw,
        tc.tile_pool(name="x", bufs=2) as px,
        tc.tile_pool(name="qkv", bufs=2) as pqkv,
        tc.tile_pool(name="rl", bufs=1) as prl,
        tc.tile_pool(name="o", bufs=2) as po,
        tc.tile_pool(name="psA", bufs=bufsA, space="PSUM") as psA,
        tc.tile_pool(name="psO", bufs=bufsO, space="PSUM") as psO,
    ):
        psS = psA
        psG = psO
        alt = _Alt(nc, dve_share=dve_share)
        import contextlib
        loop_ctx = tc.For_i(0, loop_repeat, 1) if loop_repeat > 1 else contextlib.nullcontext()

        # ---- weights, loaded once: [128 k-part, KT k-tile, 768 out-col] ----
        # On the ACT HWDGE queue (X tiles go via SP) so neither waits on the
        # other; per-weight split (k0 | k1..5) so the first matmul can start
        # as soon as the first chunk lands.
        wq = pw.tile([128, KT, D], BF16, tag="wq")
        wk = pw.tile([128, KT, D], BF16, tag="wk")
        wv = pw.tile([128, KT, D], BF16, tag="wv")
        for wt, wd in ((wq, wq_d), (wk, wk_d), (wv, wv_d)):
            nc.scalar.dma_start(
                out=wt[:, 0, :], in_=wd[0:128, :])
            nc.scalar.dma_start(
                out=wt[:, 1:KT, :],
                in_=wd[128:D, :].rearrange("(kt k) n -> k kt n", k=128))

        with loop_ctx:
         for b in [bb for _ in range(repeat) for bb in range(b_pc)]:
            # ---- X^T tiles (host pre-transposed): [128 k, KT, 640 s] bf16 ----
            # one tile, two DMAs (k0 | k1..5) on the SP HWDGE queue
            xts = px.tile([128, KT, S_PAD], BF16, tag="xts")
            nc.sync.dma_start(out=xts[:, 0, :], in_=hs[b, 0:128, :])
            nc.sync.dma_start(
                out=xts[:, 1:KT, :],
                in_=hs[b, 128:D, :].rearrange("(kt k) s -> k kt s", k=128))

            # ---- qT: 6 head-paired tiles [128, S] bf16 (head h -> tile h//2, half h%2)
            qts = []
            for m in (range(KT) if stage >= 1 else []):
                ps = psA.tile([128, S], F32, tag="big")
                for k in range(KT):
                    for s0, w in N_CHUNKS:
                        nc.tensor.matmul(ps[:, s0:s0 + w],
                                         wq[:, k, 128 * m:128 * (m + 1)],
                                         xts[:, k, s0:s0 + w],
                                         start=(k == 0), stop=(k == KT - 1))
                qt = pqkv.tile([128, S], BF16, tag=f"qt{m}")
                alt.copy(qt[:, :], ps[:, :])
                qts.append(qt)

            # ---- kT + scores + V, interleaved per head-pair ----
            # Emitting scores[m] right after kT[m] (with a V t-tile between
            # rounds) spreads the big relu evicts across the whole projection
            # span instead of concentrating them in an evict-bound tail.
            kts = {}   # m -> [128, S] bf16 tile
            vts = [None] * NT
            rls = {}   # (r, t) -> relu'd scoresT tile [t-part, s-free]

            def emit_kt(m):
                ps = psA.tile([128, S], F32, tag="big")
                for k in range(KT):
                    for s0, w in N_CHUNKS:
                        nc.tensor.matmul(ps[:, s0:s0 + w],
                                         wk[:, k, 128 * m:128 * (m + 1)],
                                         xts[:, k, s0:s0 + w],
                                         start=(k == 0), stop=(k == KT - 1))
                kt = pqkv.tile([128, S], BF16, tag=f"kt{m}")
                alt.copy(kt[:, :], ps[:, :])
                kts[m] = kt

            def emit_v(t):
                ps = psA.tile([128, D], F32, tag="big")
                for k in range(KT):
                    for c0, cw in ((0, 512), (512, 256)):
                        nc.tensor.matmul(ps[:, c0:c0 + cw],
                                         xts[:, k, 128 * t:128 * (t + 1)],
                                         wv[:, k, c0:c0 + cw],
                                         start=(k == 0), stop=(k == KT - 1))
                vt = px.tile([128, H, HD + 2], BF16, tag=f"vt{t}")
                alt.copy(vt[:, :, 0:HD], ps[:].rearrange("p (h d) -> p h d", d=HD))
                nc.gpsimd.memset(vt[:, :, HD:HD + 1], 1.0)
                vts[t] = vt

            def emit_scores(r):
                h = nu + r
                m, half = h // 2, h % 2
                for t in range(NT):
                    t0, tw = 128 * t, min(128, S - 128 * t)
                    ps = psS.tile([128, S], F32, tag="big")
                    for s0, w in N_CHUNKS:
                        nc.tensor.matmul(ps[0:tw, s0:s0 + w],
                                         kts[m][64 * half:64 * half + 64, t0:t0 + tw],
                                         qts[m][64 * half:64 * half + 64, s0:s0 + w],
                                         start=True, stop=True)
                    rl = prl.tile([tw, S], BF16, tag=f"rl{r}_{t}", bufs=rlbufs)
                    if split_relu:
                        nc.vector.tensor_scalar_max(rl[0:tw, 0:320], ps[0:tw, 0:320], 0.0)
                        nc.scalar.activation(rl[0:tw, 320:S], ps[0:tw, 320:S], Relu)
                    else:
                        alt.relu(rl[0:tw, :], ps[0:tw, :])
                    rls[(r, t)] = rl

            def emit_scores_pair(rpair):
                """Both halves of one head-pair, alternating per-MM so the
                two 64-row groups execute concurrently on the PE array."""
                m = (nu + rpair[0]) // 2
                for t in range(NT):
                    t0, tw = 128 * t, min(128, S - 128 * t)
                    pss = {}
                    for r in rpair:
                        pss[r] = psS.tile([128, S], F32, tag="big",
                                          name=f"scp{r}_{t}")
                    for s0, w in N_CHUNKS:
                        for r in rpair:
                            half = (nu + r) % 2
                            nc.tensor.matmul(pss[r][0:tw, s0:s0 + w],
                                             kts[m][64 * half:64 * half + 64, t0:t0 + tw],
                                             qts[m][64 * half:64 * half + 64, s0:s0 + w],
                                             start=True, stop=True)
                    for ri, r in enumerate(rpair):
                        rl = prl.tile([tw, S], BF16, tag=f"rl{r}_{t}", bufs=rlbufs)
                        alt.relu(rl[0:tw, :], pss[r][0:tw, :])
                        rls[(r, t)] = rl

            if stage >= 1:
                if ilv:
                    vq = 0   # next V t-tile to emit
                    for m in kt_mtiles:
                        emit_kt(m)
                        if stage >= 3:
                            for r in [2 * m - nu, 2 * m + 1 - nu]:
                                if 0 <= r < nr:
                                    emit_scores(r)
                        if vq < NT:
                            emit_v(vq)
                            vq += 1
                    while vq < NT:
                        emit_v(vq)
                        vq += 1
                else:
                    for m in kt_mtiles:
                        emit_kt(m)
                    for t in range(NT):
                        emit_v(t)
                    if stage >= 3:
                        if pair:
                            r = 0
                            while r < nr:
                                if (r + 1 < nr and (nu + r) % 2 == 0
                                        and (nu + r + 1) // 2 == (nu + r) // 2):
                                    emit_scores_pair([r, r + 1])
                                    r += 2
                                else:
                                    emit_scores(r)
                                    r += 1
                        else:
                            for r in range(nr):
                                emit_scores(r)

            # ---- K natural for uniform heads (per group) + G = K^T V / S ----
            g_tiles = {}   # u -> (tile, half) ; paired like qT parity
            for grp in (ugroups if stage >= 2 else []):
                gw = 64 * len(grp)
                knats = []
                for t in range(NT):
                    ps = psO.tile([128, 512], F32, tag="O")
                    for k in range(KT):
                        nc.tensor.matmul(ps[:, 0:gw],
                                         xts[:, k, 128 * t:128 * (t + 1)],
                                         wk[:, k, 64 * grp[0]:64 * grp[0] + gw],
                                         start=(k == 0), stop=(k == KT - 1))
                    kn = pqkv.tile([128, 384], BF16, tag=f"kn{t}")
                    alt.copy(kn[:, 0:gw], ps[:, 0:gw])
                    knats.append(kn)
                for ui, u in enumerate(grp):
                    half = u % 2
                    gp = psG.tile([128, 512], F32, tag="O")
                    for t in range(NT):
                        nc.tensor.matmul(gp[64 * half:64 * half + 64, 0:HD],
                                         knats[t][:, 64 * ui:64 * ui + 64],
                                         vts[t][:, u, 0:HD],
                                         start=(t == 0), stop=(t == NT - 1))
                    gt = po.tile([128, HD], BF16, tag=f"g{u // 2}")
                    alt.scale_const(gt[64 * half:64 * half + 64, :],
                                    gp[64 * half:64 * half + 64, 0:HD], 1.0 / S)
                    g_tiles[u] = gt

            # ---- uniform outputs early (off the tail critical path) ----
            # merged strided evict per (parity, s-tile): dest head-blocks are
            # stride-128 in processed order (positions par, par+2, ...)
            stgs = {}
            if stage >= 5:
                for si, (s0, sw) in enumerate(S_CHUNKS):
                    stg = po.tile([128, D], BF16, tag=f"st{si}")
                    stgs[si] = stg
                    for grp in ugroups:
                        for par in (0, 1):
                            pgrp = [u for u in grp if u % 2 == par]
                            if not pgrp:
                                continue
                            op = psO.tile([128, 512], F32, tag="O")
                            for ui, u in enumerate(pgrp):
                                nc.tensor.matmul(op[0:sw, 64 * ui:64 * ui + 64],
                                                 qts[u // 2][64 * par:64 * par + 64, s0:s0 + sw],
                                                 g_tiles[u][64 * par:64 * par + 64, :],
                                                 start=True, stop=True)
                            npg = len(pgrp)
                            # contiguous-in-processed-order check (u = par, par+2, ..)
                            if all(pgrp[i] == par + 2 * i for i in range(npg)):
                                dest = stg[0:sw, :].rearrange(
                                    "p (g c) -> p g c", c=64)[:, par:par + 2 * npg:2, :]
                                alt.copy(dest,
                                         op[0:sw, 0:64 * npg].rearrange(
                                             "p (g c) -> p g c", c=64))
                            else:
                                for ui, u in enumerate(pgrp):
                                    alt.copy(stg[0:sw, 64 * u:64 * u + HD],
                                             op[0:sw, 64 * ui:64 * ui + 64])

            # ---- relu outputs per s-tile (uniform already in stgs) ----
            for si, (s0, sw) in enumerate(S_CHUNKS if stage >= 4 else []):
                stg = stgs[si]
                for gi, grp in enumerate(rgroups):
                    op = psO.tile([128, 512], F32, tag="O")
                    for ri, r in enumerate(grp):
                        h = nu + r
                        for t in range(NT):
                            kk = 128 if t < NT - 1 else S - 512
                            nc.tensor.matmul(op[0:sw, 65 * ri:65 * ri + 65],
                                             rls[(r, t)][0:kk, s0:s0 + sw],
                                             vts[t][0:kk, h, 0:HD + 1],
                                             start=(t == 0), stop=(t == NT - 1))
                    rec = po.tile([128, 8], F32, tag=f"rec{gi}")
                    ng = len(grp)
                    op3 = op[0:sw, 0:65 * ng].rearrange("p (r c) -> p r c", c=65)
                    nc.vector.tensor_scalar_add(rec[0:sw, 0:ng], op3[:, :, HD], EPS)
                    nc.vector.reciprocal(rec[0:sw, 0:ng], rec[0:sw, 0:ng])
                    g0 = 64 * (nu + grp[0])
                    # single broadcast multiply on DVE
                    rec_bc = rec[0:sw, 0:ng].unsqueeze(2).broadcast_to((sw, ng, 64))
                    nc.vector.tensor_mul(
                        stg[0:sw, g0:g0 + 64 * ng].rearrange("p (r c) -> p r c", c=64),
                        op3[:, :, 0:HD], rec_bc)

                nc.sync.dma_start(out=out_d[b, s0:s0 + sw, :], in_=stg[0:sw, :])

    nc.compile()
    return nc


_CACHE = {}


def _get_nc(mask, b_pc=B_PC):
    key = (mask, b_pc)
    if key not in _CACHE:
        _CACHE[key] = build(mask, b_pc)
    return _CACHE[key]


def prep_inputs(hidden_states, Wq, Wk, Wv, mask):
    """Host-side prep: head permutation, 1/sqrt(hd) fold, bf16 cast, S pad,
    and the X -> X^T transpose (device gets [b, D, S_PAD])."""
    uniform = [h for h in range(H) if not mask[h]]
    relu_heads = [h for h in range(H) if mask[h]]
    perm = uniform + relu_heads
    cols = np.concatenate([np.arange(64 * h, 64 * h + 64) for h in perm])
    wq_p = np.ascontiguousarray(
        (np.asarray(Wq, np.float32)[:, cols] * 0.125).astype(ml_dtypes.bfloat16))
    wk_p = np.ascontiguousarray(
        np.asarray(Wk, np.float32)[:, cols].astype(ml_dtypes.bfloat16))
    wv_p = np.ascontiguousarray(
        np.asarray(Wv, np.float32)[:, cols].astype(ml_dtypes.bfloat16))
    hsf = np.asarray(hidden_states, np.float32)
    hs_p = np.zeros((hsf.shape[0], D, S_PAD), ml_dtypes.bfloat16)
    hs_p[:, :, :S] = hsf.astype(ml_dtypes.bfloat16).transpose(0, 2, 1)
    return hs_p, wq_p, wk_p, wv_p


def make_in_maps(hidden_states, Wq, Wk, Wv, mask, b_pc=B_PC):
    hs_p, wq_p, wk_p, wv_p = prep_inputs(hidden_states, Wq, Wk, Wv, mask)
    n_shards = hs_p.shape[0] // b_pc
    return [
        {"hs": hs_p[c * b_pc:(c + 1) * b_pc], "wq": wq_p, "wk": wk_p, "wv": wv_p}
        for c in range(n_shards)
    ]


def unpermute_out(out_p, mask):
    """Device output is bf16 in processed head order; un-permute + fp32."""
    uniform = [h for h in range(H) if not mask[h]]
    relu_heads = [h for h in range(H) if mask[h]]
    perm = uniform + relu_heads
    out = np.empty(out_p.shape, np.float32)
    o3 = out.reshape(*out_p.shape[:-1], H, HD)
    p3 = np.asarray(out_p).reshape(*out_p.shape[:-1], H, HD)
    for p, horig in enumerate(perm):
        o3[..., horig, :] = p3[..., p, :].astype(np.float32)
    return out


def kernel(hidden_states, Wq, bq, Wk, bk, Wv, bv, head_mask, layer_count=None, **_):
    for bias in (bq, bk, bv):
        assert not np.any(np.asarray(bias)), "nonzero qkv biases unsupported"
    mask = tuple(bool(x) for x in np.asarray(head_mask).reshape(-1))
    assert len(mask) == H

    nc = _get_nc(mask)
    in_maps = make_in_maps(hidden_states, Wq, Wk, Wv, mask)
    res = run_bass_kernel_spmd(nc, in_maps, list(range(N_CORES)))
    out_p = np.concatenate([res.results[c]["out"] for c in range(N_CORES)], axis=0)
    return np.ascontiguousarray(unpermute_out(out_p, mask))
